# revision 52
# baseline (speedup 1.0000x reference)
"""Trainium2 Bass kernel: 16-head MHA (B=2, S=2048, E=1024) on 8 NeuronCores.

Sharding: core c = (batch b = c // 4, head-group g = c % 4); each core runs
4 heads of one batch (data parallel on B x tensor parallel on heads).  The
output projection is row-sharded: each core produces a partial [S, E] f32
output; the host sums the 4 head-group partials per batch and adds bo.

Device schedule: a single chunk-pipelined loop over the four 512-column
m-chunks.  Per chunk c the PE stream is
    proj(c):   k,q chains (weight-stationary, 8 e-tile PSUM chains)
    v(c):      v_aug computed DIRECTLY in [n, dv] layout (x-tile stationary,
               wv moving) -- no PE transposes; interleaved with
    outproj(c-1): per (m-tile, e-half) chains over both pairs
    attn(c):   scoresT (kT stationary, K=64 row-group packed), exp on ACT,
               0/1 triangle multiply on diagonal tiles (gpsimd), attnV
               (v_aug stationary) software-pipelined two steps behind
so the ACT-bound attention of chunk c overlaps the PE-only projection of
chunk c+1 and the output projection of chunk c-1.  The softmax denominator
(from the ones-column of v_aug) takes a single DRAM bounce: written [1,MC],
re-read with a step-0 partition AP as a [64,MC] broadcast, reciprocal via
the fast custom-DVE approx, then one DVE multiply into oT.  Input DMAs are
issued e-tile-granular across the sync/scalar/vector queues in consumption
order so the k-projection starts ~2us in.
"""

import numpy as np
import ml_dtypes

B, S, E = 2, 2048, 1024
H, DK = 16, 64
NCORES = 8
G = 4                 # head-groups (tensor parallel degree)
NH = H // G           # heads per core = 4
DKH = NH * DK         # 256 head dims per core
P = 128
MC = 512              # m-chunk (psum bank width in f32)
NMC = S // MC         # 4 m-chunks
NT = S // P           # 16 n-tiles (and m-tiles)
ET = E // P           # 8 e-tiles
PAIRS = NH // 2       # 2 head pairs per core
BF16 = ml_dtypes.bfloat16
SCALE = float(1.0 / np.sqrt(np.float32(DK)))


def _build_program(chunk_ntiles, causal, bias_qk, bias_v):
    """Build the (SPMD, shared across all 8 cores) Bass program.

    chunk_ntiles[c] = number of 128-wide n-tiles to process for m-chunk c.
    causal: apply diagonal-tile masking (memset + tri multiply).
    """
    from contextlib import ExitStack

    import concourse.bass as bass
    import concourse.tile as tile
    from concourse import bacc, mybir

    f32 = mybir.dt.float32
    bf16 = mybir.dt.bfloat16
    Exp = mybir.ActivationFunctionType.Exp

    nc = bacc.Bacc(
        "TRN2",
        target_bir_lowering=False,
        debug=False,
        enable_asserts=False,
        num_devices=NCORES,
    )

    # ---- DRAM I/O ----
    # xq blocked by m-chunk, xv blocked by pairs of 128-wide n-tiles: the
    # host pre-permutes so each block is one contiguous-per-partition DMA
    # and lands exactly when the pipeline first needs it.
    xqB = nc.dram_tensor("xqB", [NMC, P, ET * MC], bf16, kind="ExternalInput").ap()
    xkT = nc.dram_tensor("xkT", [E, S], bf16, kind="ExternalInput").ap()
    xvB = nc.dram_tensor("xvB", [ET, P, ET * 2 * P], bf16, kind="ExternalInput").ap()
    wkqvT = nc.dram_tensor("wkqvT", [E, 3 * DKH], bf16, kind="ExternalInput").ap()
    woT = nc.dram_tensor("woT", [DKH, E], bf16, kind="ExternalInput").ap()
    dmask = nc.dram_tensor("dmask", [P, P], bf16, kind="ExternalInput").ap()
    if bias_qk:
        bqd = nc.dram_tensor("bq", [DKH, 1], f32, kind="ExternalInput").ap()
        bkd = nc.dram_tensor("bk", [DKH, 1], f32, kind="ExternalInput").ap()
    if bias_v:
        bvd = nc.dram_tensor("bv", [1, DKH], f32, kind="ExternalInput").ap()
    # bf16 partials: halves output DMA + DVE eviction bytes; the host sums
    # the four head-group partials per batch in f32
    out = nc.dram_tensor("out", [S, E], bf16, kind="ExternalOutput").ap()

    with tile.TileContext(nc) as tc, ExitStack() as ctx:
        const = ctx.enter_context(tc.tile_pool(name="const", bufs=1))
        xpool = ctx.enter_context(tc.tile_pool(name="xpool", bufs=1))
        wpool = ctx.enter_context(tc.tile_pool(name="wpool", bufs=1))
        qkpool = ctx.enter_context(tc.tile_pool(name="qkpool", bufs=1))
        vpool = ctx.enter_context(tc.tile_pool(name="vpool", bufs=1))
        prpool = ctx.enter_context(tc.tile_pool(name="prpool", bufs=8))
        rcpool = ctx.enter_context(tc.tile_pool(name="rcpool", bufs=2))
        otpool = ctx.enter_context(tc.tile_pool(name="otpool", bufs=1))
        ostpool = ctx.enter_context(tc.tile_pool(name="ostpool", bufs=4))
        # PSUM: "pj" (proj + outproj chains) 2 banks, "sc" 4 banks,
        # "oaug" 2 banks -- exactly the 8 banks.
        pjps = ctx.enter_context(tc.tile_pool(name="pj_ps", bufs=2, space="PSUM"))
        scps = ctx.enter_context(tc.tile_pool(name="sc_ps", bufs=2, space="PSUM"))
        oaps = ctx.enter_context(tc.tile_pool(name="oa_ps", bufs=2, space="PSUM"))
        rcdram = ctx.enter_context(tc.tile_pool(name="rc_dram", bufs=4, space="DRAM"))

        # ---- persistent SBUF tiles ----
        wkqv_sb = wpool.tile([P, ET, 3 * DKH], bf16, tag="wkqv")
        wo_sb = wpool.tile([P, PAIRS, E], bf16, tag="wo")
        xk_t = xpool.tile([P, ET, S], bf16, tag="xk")
        xq_t = xpool.tile([P, ET, S], bf16, tag="xq")
        xv_t = xpool.tile([P, ET, S], bf16, tag="xv")
        dmask_sb = const.tile([P, P], bf16, tag="dmask")

        # ---- input DMA issue.  Three concurrent queues (sync/scalar HWDGE
        # + gpsimd SWDGE) each sustain ~HBM/3; transfers are large (fewer
        # per-transfer gaps) and ordered by first consumption:
        # weights -> xk -> xq chunk 0 -> xv n-blocks 0-3 -> xq 1.. -> xv 4..
        def wk_slab(a, b):
            return (wkqv_sb[:, a:b, :], wkqvT[P * a : P * b, :].rearrange(
                "(t p) o -> p t o", p=P))

        def xk_slab(a, b):
            src = xkT.rearrange("(t p) s -> p t s", p=P)[:, a:b, :]
            return (xk_t[:, a:b, :], src)

        def xq_chunk(c):
            src = xqB[c].rearrange("p (t n) -> p t n", t=ET)
            return (xq_t[:, :, MC * c : MC * (c + 1)], src)

        # scalar issues NOTHING (dma_start costs the issuing engine
        # ~8.6ns/descriptor on HWDGE queues -- it must stay free for exp);
        # gpsimd's SWDGE issue is ~3x cheaper per descriptor.
        def xv_block(nb):
            src = xvB[nb].rearrange("p (t n) -> p t n", t=ET)
            return (xv_t[:, :, 2 * P * nb : 2 * P * (nb + 1)], src)

        # Queue roles (measured): gpsimd's SWDGE sustains ~224GB/s with cheap
        # issue -- it carries the bulk, in consumption order.  sync/scalar
        # HWDGE queues burst fast only while SWDGE ramps, and DMA execution
        # is IN-ORDER per queue -- so sync carries only the early-burst
        # slice and then stays clear for the latency-critical denominator
        # and output DMAs; scalar (exp engine) gets a single early tile.
        sync_q = [
            wk_slab(0, 2),
            xk_slab(0, 1),
            wk_slab(2, 4),
            xk_slab(2, 3),
            xk_slab(4, 5),
        ]
        scalar_q = [xk_slab(6, 7), xk_slab(7, 8)]
        gpsimd_q = (
            [(dmask_sb, dmask), wk_slab(4, 6), xk_slab(1, 2), wk_slab(6, 8)]
            + [xk_slab(i, i + 1) for i in (3, 5)]
            + [
                xq_chunk(0),
                xv_block(0),
                xv_block(1),
                xq_chunk(1),
                xv_block(2),
                xv_block(3),
                xq_chunk(2),
                (wo_sb[:, 0, :], woT[0:P, :]),
                (wo_sb[:, 1, :], woT[P : 2 * P, :]),
                xq_chunk(3),
                xv_block(4),
                xv_block(5),
                xv_block(6),
                xv_block(7),
            ]
        )
        for eng, q in ((nc.sync, sync_q), (nc.scalar, scalar_q), (nc.gpsimd, gpsimd_q)):
            for dst, src in q:
                eng.dma_start(out=dst, in_=src)

        if bias_qk:
            bq_sb = const.tile([P, PAIRS], f32, tag="bq")
            nc.sync.dma_start(out=bq_sb, in_=bqd.rearrange("(t p) o -> p (t o)", p=P))
            bk_sb = const.tile([P, PAIRS], f32, tag="bk")
            nc.sync.dma_start(out=bk_sb, in_=bkd.rearrange("(t p) o -> p (t o)", p=P))
        if bias_v:
            # bv along the free dim of the direct v_aug layout: broadcast
            # [1, DKH] across all partitions via a step-0 partition AP.
            bv_sb = const.tile([P, DKH], f32, tag="bv")
            nc.sync.dma_start(
                out=bv_sb,
                in_=bass.AP(
                    tensor=bvd.tensor,
                    offset=bvd.offset,
                    ap=[[0, P]] + [list(a) for a in bvd.ap[1:]],
                ),
            )

        # persistent activation tiles
        qT_sb = [qkpool.tile([P, S], bf16, tag=f"qT{p}", name=f"qT_sb{p}") for p in range(PAIRS)]
        kT_sb = [qkpool.tile([P, S], bf16, tag=f"kT{p}", name=f"kT_sb{p}") for p in range(PAIRS)]
        vaug_sb = [vpool.tile([P, NH, DK + 1], bf16, tag=f"va{j}", name=f"vaug_sb{j}") for j in range(NT)]
        oT_sb = [otpool.tile([P, S], bf16, tag=f"oT{p}", name=f"oT_sb{p}") for p in range(PAIRS)]

        wv0 = 2 * DKH

        def proj_chain(dst, w0, x_t, bias, p, c):
            ps = pjps.tile([P, MC], f32, tag="pj", name="ps_kq")
            for i in range(ET):
                nc.tensor.matmul(
                    ps,
                    wkqv_sb[:, i, w0 + P * p : w0 + P * (p + 1)],
                    x_t[:, i, MC * c : MC * (c + 1)],
                    start=(i == 0),
                    stop=(i == ET - 1),
                )
            dslice = dst[p][:, MC * c : MC * (c + 1)]
            if bias is not None:
                nc.vector.tensor_scalar_add(dslice, ps, bias[:, p : p + 1])
            else:
                nc.vector.tensor_copy(dslice, ps)

        def v_chain(j):
            # v_aug[n-tile j] = xv-tile stationary, wv moving: [128 n, 256 dv]
            ps = pjps.tile([P, MC], f32, tag="pj", name="ps_v")
            pv = ps[:, 0:DKH]
            for i in range(ET):
                nc.tensor.matmul(
                    pv,
                    xv_t[:, i, P * j : P * (j + 1)],
                    wkqv_sb[:, i, wv0 : wv0 + DKH],
                    start=(i == 0),
                    stop=(i == ET - 1),
                )
            if bias_v:
                nc.vector.tensor_add(pv, pv, bv_sb)
            pt3 = pv.rearrange("n (h d) -> n h d", h=NH)
            nc.vector.tensor_copy(vaug_sb[j][:, :, 0:DK], pt3)
            nc.gpsimd.memset(vaug_sb[j][:, :, DK : DK + 1], 1.0)

        def op_chain(t, ec, evict_eng, dma_eng, pool_tag=None):
            pool, tag = pool_tag or (pjps, "pj")
            op = pool.tile([P, MC], f32, tag=tag, name="op_t")
            for p in range(PAIRS):
                nc.tensor.matmul(
                    op,
                    oT_sb[p][:, P * t : P * (t + 1)],
                    wo_sb[:, p, MC * ec : MC * (ec + 1)],
                    start=(p == 0),
                    stop=(p == PAIRS - 1),
                )
            ost = ostpool.tile([P, MC], bf16, tag="ost", name="ost_t")
            if evict_eng is nc.scalar:
                nc.scalar.copy(ost, op)
            else:
                evict_eng.tensor_copy(ost, op)
            dma_eng.dma_start(
                out=out[P * t : P * (t + 1), MC * ec : MC * (ec + 1)], in_=ost
            )

        def attn_group(c, p, fillers):
            """fillers: list of zero-arg callables emitting PE filler chains
            (v_aug builds, outproj chains); one is consumed right before
            each attnV step so the PE has work while ACT runs exp."""
            J = chunk_ntiles[c]
            oaug = [
                oaps.tile([P, MC], f32, tag="oaug", name=f"oaug{h01}")
                for h01 in range(2)
            ]
            probs_tiles = [None] * J

            def scores_step(j):
                # columns left of `off` in this m-chunk are fully masked
                # for n-tile j: never compute/exp/consume them
                off = P * (j - 4 * c) if (causal and j >= 4 * c) else 0
                sc = scps.tile([P, 2 * MC], f32, tag="sc", name="sc_ps_t")
                for h01 in range(2):
                    nc.tensor.matmul(
                        sc[:, MC * h01 + off : MC * (h01 + 1)],
                        kT_sb[p][64 * h01 : 64 * (h01 + 1), P * j : P * (j + 1)],
                        qT_sb[p][64 * h01 : 64 * (h01 + 1), MC * c + off : MC * (c + 1)],
                        start=True,
                        stop=True,
                    )
                probs = prpool.tile([P, 2 * MC], bf16, tag="probs", name="probs_t")
                sc3 = sc.rearrange("p (u m) -> p u m", u=2)
                pr3 = probs.rearrange("p (u m) -> p u m", u=2)
                nc.scalar.activation(
                    pr3[:, :, off:MC], sc3[:, :, off:MC], Exp, bias=0.0, scale=SCALE
                )
                if causal and j >= 4 * c:
                    for h01 in range(2):
                        base = MC * h01 + off
                        nc.gpsimd.tensor_mul(
                            probs[:, base : base + P],
                            probs[:, base : base + P],
                            dmask_sb,
                        )
                probs_tiles[j] = (probs, off)

            def attnv_step(j):
                probs, off = probs_tiles[j]
                for h01 in range(2):
                    h = 2 * p + h01
                    nc.tensor.matmul(
                        oaug[h01][0 : DK + 1, off:MC],
                        vaug_sb[j][:, h, :],
                        probs[:, MC * h01 + off : MC * (h01 + 1)],
                        start=(j == 0),
                        stop=(j == J - 1),
                    )

            # software pipeline: scores two steps ahead of attnV, with one
            # PE filler chain in front of each attnV step
            def take_filler():
                if fillers:
                    fillers.pop(0)()

            # depth-3 pipeline: attnV trails scores by 3 steps so it never
            # waits on the exp semaphore (scores themselves are 2-limited
            # by the sc PSUM rotation)
            for j in range(J):
                scores_step(j)
                if j >= 3:
                    take_filler()
                    attnv_step(j - 3)
            for j in range(max(0, J - 3), J):
                take_filler()
                attnv_step(j)

            # epilogue: evict o_aug to SBUF (frees PSUM); bounce the
            # denominator row once through DRAM to re-read it broadcast
            # across 64 partitions (step-0 partition AP), fast-reciprocal,
            # multiply into oT.
            # last group's bounce goes through scalar (idle after the final
            # exp) so it never queues behind out-DMAs on sync
            den_eng = nc.scalar if (c == NMC - 1 and p == PAIRS - 1) else nc.sync
            osb = []
            for h01 in range(2):
                o = rcpool.tile([DK + 1, MC], f32, tag="osb", bufs=4, name="osb_t")
                nc.vector.tensor_copy(o, oaug[h01][0 : DK + 1, :])
                den_d = rcdram.tile([1, MC], f32, tag="den_d", name="den_d_t")
                den_eng.dma_start(out=den_d, in_=o[DK : DK + 1, :])
                bcden = rcpool.tile([64, MC], f32, tag="bc", bufs=4, name="bc_t")
                den_eng.dma_start(
                    out=bcden,
                    in_=bass.AP(
                        tensor=den_d.tensor,
                        offset=den_d.offset,
                        ap=[[0, 64]] + [list(a) for a in den_d.ap[1:]],
                    ),
                )
                osb.append((o, bcden))
            for h01 in range(2):
                o, bcden = osb[h01]
                rc = rcpool.tile([64, MC], f32, tag="rc", bufs=2, name="rc_t")
                nc.vector.reciprocal_approx_fast(out=rc, in_=bcden)
                nc.vector.tensor_mul(
                    oT_sb[p][64 * h01 : 64 * (h01 + 1), MC * c : MC * (c + 1)],
                    o[0:DK, :],
                    rc,
                )

        # ---- schedule ----
        # Chunk-pipelined: k/q chains for chunk c+1 are emitted BETWEEN
        # attention windows (their DVE evictions land before the windows'
        # deferred epilogues, so they never queue behind a bounce DMA).
        # v_aug builds and the previous chunk's outproj chains ride inside
        # the attention groups as PE fillers, one per attnV step.
        kb = bk_sb if bias_qk else None
        qb = bq_sb if bias_qk else None

        def kq(c):
            for p in range(PAIRS):
                proj_chain(kT_sb, 0, xk_t, kb, p, c)
            for p in range(PAIRS):
                proj_chain(qT_sb, DKH, xq_t, qb, p, c)

        if causal:
            kq(0)
        else:
            # non-causal: attn(0) consumes every kT tile, so all k/q must
            # precede it in the in-order PE stream
            for c in range(NMC):
                kq(c)
        for c in range(NMC):
            if causal:
                vjs = list(range(4 * c, 4 * c + 4))
            else:
                vjs = list(range(NT)) if c == 0 else []
            fillers = [
                (lambda j=j: v_chain(j)) for j in vjs
            ]
            if c > 0:
                fillers += [
                    (lambda t=t, ec=ec: op_chain(t, ec, nc.vector, nc.sync))
                    for t in range(4 * (c - 1), 4 * c)
                    for ec in range(E // MC)
                ]
            if causal and c + 1 < NMC:
                # next chunk's k/q chains ride as late fillers too: exp(0)
                # starts ~20us earlier than with a full k/q prologue
                fillers += [
                    (lambda p=p, c1=c + 1: proj_chain(kT_sb, 0, xk_t, kb, p, c1))
                    for p in range(PAIRS)
                ]
                fillers += [
                    (lambda p=p, c1=c + 1: proj_chain(qT_sb, DKH, xq_t, qb, p, c1))
                    for p in range(PAIRS)
                ]
            for p in range(PAIRS):
                attn_group(c, p, fillers)
            for f in fillers:
                f()
            del fillers[:]

        # tail: outproj of the last chunk.  ACT is idle now (scalar evicts
        # half); chains alternate between the pj and oaug PSUM pools so four
        # banks rotate instead of two.
        n = 0
        tail_dma = (nc.sync, nc.gpsimd, nc.scalar)
        for t in range(4 * (NMC - 1), 4 * NMC):
            for ec in range(E // MC):
                op_chain(
                    t,
                    ec,
                    nc.scalar if n % 2 == 0 else nc.vector,
                    tail_dma[n % 3],
                    pool_tag=(pjps, "pj") if n % 2 == 0 else (oaps, "oaug"),
                )
                n += 1

    nc.compile()
    return nc


def _host_inputs(key, value, query, Wk, Wq, Wv, Wo, bq, bk, bv, bias_qk, bias_v):
    """Per-core input maps (host-side shard/transpose/cast — not timed)."""
    tri = np.triu(np.ones((P, P), np.float32)).astype(BF16)  # allowed: n<=m
    in_maps = []
    xT = {}
    for b in range(B):
        xqT = query[b].T.astype(BF16)  # [E, S]
        xkT = np.ascontiguousarray(key[b].T).astype(BF16)
        xvT = value[b].T.astype(BF16)
        # xqB[c, p, t*MC+n] = xqT[t*P+p, c*MC+n]  (m-chunk blocked)
        xT[("qB", b)] = np.ascontiguousarray(
            xqT.reshape(ET, P, NMC, MC).transpose(2, 1, 0, 3).reshape(NMC, P, ET * MC)
        )
        # xvB[nb, p, t*256+n] = xvT[t*P+p, nb*256+n]  (n-block blocked)
        xT[("vB", b)] = np.ascontiguousarray(
            xvT.reshape(ET, P, ET, 2 * P).transpose(2, 1, 0, 3).reshape(ET, P, ET * 2 * P)
        )
        xT[("k", b)] = xkT
    for c in range(NCORES):
        b, g = divmod(c, G)
        sl = slice(DKH * g, DKH * (g + 1))
        wkqv = np.concatenate(
            [Wk[sl].T, Wq[sl].T, Wv[sl].T], axis=1
        )  # [E, 3*DKH], column blocks K|Q|V
        m = {
            "xqB": xT[("qB", b)],
            "xkT": xT[("k", b)],
            "xvB": xT[("vB", b)],
            "wkqvT": np.ascontiguousarray(wkqv).astype(BF16),
            "woT": np.ascontiguousarray(Wo[:, sl].T).astype(BF16),
            "dmask": tri,
        }
        if bias_qk:
            m["bq"] = np.ascontiguousarray(bq[sl].astype(np.float32).reshape(DKH, 1))
            m["bk"] = np.ascontiguousarray(bk[sl].astype(np.float32).reshape(DKH, 1))
        if bias_v:
            m["bv"] = np.ascontiguousarray(bv[sl].astype(np.float32).reshape(1, DKH))
        in_maps.append(m)
    return in_maps


def _numpy_fallback(key, value, query, mask, Wk, bk, Wq, bq, Wv, bv, Wo, bo):
    """Exact reference semantics in numpy (general-mask fallback)."""
    def proj(x, W, b):
        return x @ W.T + b

    k = proj(key, Wk, bk).reshape(B, S, H, DK).transpose(0, 2, 1, 3)
    q = proj(query, Wq, bq).reshape(B, S, H, DK).transpose(0, 2, 1, 3)
    v = proj(value, Wv, bv).reshape(B, S, H, DK).transpose(0, 2, 1, 3)
    scores = np.einsum("bhmd,bhnd->bhmn", q, k).astype(np.float32)
    scores = np.where(mask, scores, np.float32(-1e10)) * np.float32(SCALE)
    scores -= scores.max(axis=3, keepdims=True)
    e = np.exp(scores)
    attn = e / e.sum(axis=3, keepdims=True)
    o = np.einsum("bhmn,bhnv->bhmv", attn, v)
    o = o.transpose(0, 2, 1, 3).reshape(B, S, E)
    return (o @ Wo.T + bo).astype(np.float32)


_program_cache = {}


def kernel(key, value, query, mask, Wk, bk, Wq, bq, Wv, bv, Wo, bo):
    key = np.asarray(key, np.float32)
    value = np.asarray(value, np.float32)
    query = np.asarray(query, np.float32)
    mask = np.asarray(mask)
    Wk, bk = np.asarray(Wk, np.float32), np.asarray(bk, np.float32)
    Wq, bq = np.asarray(Wq, np.float32), np.asarray(bq, np.float32)
    Wv, bv = np.asarray(Wv, np.float32), np.asarray(bv, np.float32)
    Wo, bo = np.asarray(Wo, np.float32), np.asarray(bo, np.float32)

    m2 = mask.reshape(B, S, S) if mask.size == B * S * S else None
    causal = m2 is not None and all(
        np.array_equal(m2[b], np.tril(np.ones((S, S), bool))) for b in range(B)
    )
    allones = m2 is not None and bool(mask.all())
    if not causal and not allones:
        return _numpy_fallback(key, value, query, mask, Wk, bk, Wq, bq, Wv, bv, Wo, bo)

    if causal:
        chunk_ntiles = tuple(4 * (c + 1) for c in range(NMC))
    else:
        chunk_ntiles = tuple(NT for _ in range(NMC))

    bias_qk = bool(np.any(bq) or np.any(bk))
    bias_v = bool(np.any(bv))

    pkey = (chunk_ntiles, causal, bias_qk, bias_v)
    if pkey not in _program_cache:
        _program_cache[pkey] = _build_program(chunk_ntiles, causal, bias_qk, bias_v)
    nc = _program_cache[pkey]

    from concourse.bass_utils import run_bass_kernel_spmd

    in_maps = _host_inputs(key, value, query, Wk, Wq, Wv, Wo, bq, bk, bv, bias_qk, bias_v)
    res = run_bass_kernel_spmd(nc, in_maps, core_ids=list(range(NCORES)))

    outp = np.zeros((B, S, E), np.float32)
    for c in range(NCORES):
        outp[c // G] += np.asarray(res.results[c]["out"], np.float32)
    outp += bo.astype(np.float32)
    return outp


# revision 53
# speedup vs baseline: 1.0161x; 1.0161x over previous
"""Trainium2 Bass kernel: 16-head MHA (B=2, S=2048, E=1024) on 8 NeuronCores.

Sharding: core c = (batch b = c // 4, head-group g = c % 4); each core runs
4 heads of one batch (data parallel on B x tensor parallel on heads).  The
output projection is row-sharded: each core produces a partial [S, E] f32
output; the host sums the 4 head-group partials per batch and adds bo.

Device schedule: a single chunk-pipelined loop over the four 512-column
m-chunks.  Per chunk c the PE stream is
    proj(c):   k,q chains (weight-stationary, 8 e-tile PSUM chains)
    v(c):      v_aug computed DIRECTLY in [n, dv] layout (x-tile stationary,
               wv moving) -- no PE transposes; interleaved with
    outproj(c-1): per (m-tile, e-half) chains over both pairs
    attn(c):   scoresT (kT stationary, K=64 row-group packed), exp on ACT,
               0/1 triangle multiply on diagonal tiles (gpsimd), attnV
               (v_aug stationary) software-pipelined two steps behind
so the ACT-bound attention of chunk c overlaps the PE-only projection of
chunk c+1 and the output projection of chunk c-1.  The softmax denominator
(from the ones-column of v_aug) takes a single DRAM bounce: written [1,MC],
re-read with a step-0 partition AP as a [64,MC] broadcast, reciprocal via
the fast custom-DVE approx, then one DVE multiply into oT.  Input DMAs are
issued e-tile-granular across the sync/scalar/vector queues in consumption
order so the k-projection starts ~2us in.
"""

import numpy as np
import ml_dtypes

B, S, E = 2, 2048, 1024
H, DK = 16, 64
NCORES = 8
G = 4                 # head-groups (tensor parallel degree)
NH = H // G           # heads per core = 4
DKH = NH * DK         # 256 head dims per core
P = 128
MC = 512              # m-chunk (psum bank width in f32)
NMC = S // MC         # 4 m-chunks
NT = S // P           # 16 n-tiles (and m-tiles)
ET = E // P           # 8 e-tiles
PAIRS = NH // 2       # 2 head pairs per core
BF16 = ml_dtypes.bfloat16
SCALE = float(1.0 / np.sqrt(np.float32(DK)))


def _build_program(chunk_ntiles, causal, bias_qk, bias_v):
    """Build the (SPMD, shared across all 8 cores) Bass program.

    chunk_ntiles[c] = number of 128-wide n-tiles to process for m-chunk c.
    causal: apply diagonal-tile masking (memset + tri multiply).
    """
    from contextlib import ExitStack

    import concourse.bass as bass
    import concourse.tile as tile
    from concourse import bacc, mybir

    f32 = mybir.dt.float32
    bf16 = mybir.dt.bfloat16
    Exp = mybir.ActivationFunctionType.Exp

    nc = bacc.Bacc(
        "TRN2",
        target_bir_lowering=False,
        debug=False,
        enable_asserts=False,
        num_devices=NCORES,
    )

    # ---- DRAM I/O ----
    # xq blocked by m-chunk, xv blocked by pairs of 128-wide n-tiles: the
    # host pre-permutes so each block is one contiguous-per-partition DMA
    # and lands exactly when the pipeline first needs it.
    xqB = nc.dram_tensor("xqB", [NMC, P, ET * MC], bf16, kind="ExternalInput").ap()
    xkT = nc.dram_tensor("xkT", [E, S], bf16, kind="ExternalInput").ap()
    xvB = nc.dram_tensor("xvB", [ET, P, ET * 2 * P], bf16, kind="ExternalInput").ap()
    wkqvT = nc.dram_tensor("wkqvT", [E, 3 * DKH], bf16, kind="ExternalInput").ap()
    woT = nc.dram_tensor("woT", [DKH, E], bf16, kind="ExternalInput").ap()
    dmask = nc.dram_tensor("dmask", [P, P], bf16, kind="ExternalInput").ap()
    if bias_qk:
        bqd = nc.dram_tensor("bq", [DKH, 1], f32, kind="ExternalInput").ap()
        bkd = nc.dram_tensor("bk", [DKH, 1], f32, kind="ExternalInput").ap()
    if bias_v:
        bvd = nc.dram_tensor("bv", [1, DKH], f32, kind="ExternalInput").ap()
    # bf16 partials: halves output DMA + DVE eviction bytes; the host sums
    # the four head-group partials per batch in f32
    out = nc.dram_tensor("out", [S, E], bf16, kind="ExternalOutput").ap()

    with tile.TileContext(nc) as tc, ExitStack() as ctx:
        const = ctx.enter_context(tc.tile_pool(name="const", bufs=1))
        xpool = ctx.enter_context(tc.tile_pool(name="xpool", bufs=1))
        wpool = ctx.enter_context(tc.tile_pool(name="wpool", bufs=1))
        qkpool = ctx.enter_context(tc.tile_pool(name="qkpool", bufs=1))
        vpool = ctx.enter_context(tc.tile_pool(name="vpool", bufs=1))
        prpool = ctx.enter_context(tc.tile_pool(name="prpool", bufs=8))
        rcpool = ctx.enter_context(tc.tile_pool(name="rcpool", bufs=2))
        otpool = ctx.enter_context(tc.tile_pool(name="otpool", bufs=1))
        ostpool = ctx.enter_context(tc.tile_pool(name="ostpool", bufs=4))
        # PSUM: "pj" (proj + outproj chains) 2 banks, "sc" 4 banks,
        # "oaug" 2 banks -- exactly the 8 banks.
        pjps = ctx.enter_context(tc.tile_pool(name="pj_ps", bufs=2, space="PSUM"))
        scps = ctx.enter_context(tc.tile_pool(name="sc_ps", bufs=2, space="PSUM"))
        oaps = ctx.enter_context(tc.tile_pool(name="oa_ps", bufs=2, space="PSUM"))
        rcdram = ctx.enter_context(tc.tile_pool(name="rc_dram", bufs=4, space="DRAM"))

        # ---- persistent SBUF tiles ----
        wkqv_sb = wpool.tile([P, ET, 3 * DKH], bf16, tag="wkqv")
        wo_sb = wpool.tile([P, PAIRS, E], bf16, tag="wo")
        xk_t = xpool.tile([P, ET, S], bf16, tag="xk")
        xq_t = xpool.tile([P, ET, S], bf16, tag="xq")
        xv_t = xpool.tile([P, ET, S], bf16, tag="xv")
        dmask_sb = const.tile([P, P], bf16, tag="dmask")

        # ---- input DMA issue.  Three concurrent queues (sync/scalar HWDGE
        # + gpsimd SWDGE) each sustain ~HBM/3; transfers are large (fewer
        # per-transfer gaps) and ordered by first consumption:
        # weights -> xk -> xq chunk 0 -> xv n-blocks 0-3 -> xq 1.. -> xv 4..
        def wk_slab(a, b):
            return (wkqv_sb[:, a:b, :], wkqvT[P * a : P * b, :].rearrange(
                "(t p) o -> p t o", p=P))

        def xk_slab(a, b):
            src = xkT.rearrange("(t p) s -> p t s", p=P)[:, a:b, :]
            return (xk_t[:, a:b, :], src)

        def xq_chunk(c):
            src = xqB[c].rearrange("p (t n) -> p t n", t=ET)
            return (xq_t[:, :, MC * c : MC * (c + 1)], src)

        # scalar issues NOTHING (dma_start costs the issuing engine
        # ~8.6ns/descriptor on HWDGE queues -- it must stay free for exp);
        # gpsimd's SWDGE issue is ~3x cheaper per descriptor.
        def xv_block(nb):
            src = xvB[nb].rearrange("p (t n) -> p t n", t=ET)
            return (xv_t[:, :, 2 * P * nb : 2 * P * (nb + 1)], src)

        # Queue roles (measured): gpsimd's SWDGE sustains ~224GB/s with cheap
        # issue -- it carries the bulk, in consumption order.  sync/scalar
        # HWDGE queues burst fast only while SWDGE ramps, and DMA execution
        # is IN-ORDER per queue -- so sync carries only the early-burst
        # slice and then stays clear for the latency-critical denominator
        # and output DMAs; scalar (exp engine) gets a single early tile.
        sync_q = [
            wk_slab(0, 2),
            xk_slab(0, 1),
            wk_slab(2, 4),
            xk_slab(2, 3),
            xk_slab(4, 5),
        ]
        scalar_q = [xk_slab(6, 7), xk_slab(7, 8)]
        gpsimd_q = (
            [(dmask_sb, dmask), wk_slab(4, 6), xk_slab(1, 2), wk_slab(6, 8)]
            + [xk_slab(i, i + 1) for i in (3, 5)]
            + [
                xq_chunk(0),
                xv_block(0),
                xv_block(1),
                xq_chunk(1),
                xv_block(2),
                xv_block(3),
                xq_chunk(2),
                (wo_sb[:, 0, :], woT[0:P, :]),
                (wo_sb[:, 1, :], woT[P : 2 * P, :]),
                xq_chunk(3),
                xv_block(4),
                xv_block(5),
                xv_block(6),
                xv_block(7),
            ]
        )
        for eng, q in ((nc.sync, sync_q), (nc.scalar, scalar_q), (nc.gpsimd, gpsimd_q)):
            for dst, src in q:
                eng.dma_start(out=dst, in_=src)

        if bias_qk:
            bq_sb = const.tile([P, PAIRS], f32, tag="bq")
            nc.sync.dma_start(out=bq_sb, in_=bqd.rearrange("(t p) o -> p (t o)", p=P))
            bk_sb = const.tile([P, PAIRS], f32, tag="bk")
            nc.sync.dma_start(out=bk_sb, in_=bkd.rearrange("(t p) o -> p (t o)", p=P))
        if bias_v:
            # bv along the free dim of the direct v_aug layout: broadcast
            # [1, DKH] across all partitions via a step-0 partition AP.
            bv_sb = const.tile([P, DKH], f32, tag="bv")
            nc.sync.dma_start(
                out=bv_sb,
                in_=bass.AP(
                    tensor=bvd.tensor,
                    offset=bvd.offset,
                    ap=[[0, P]] + [list(a) for a in bvd.ap[1:]],
                ),
            )

        # persistent activation tiles
        qT_sb = [qkpool.tile([P, S], bf16, tag=f"qT{p}", name=f"qT_sb{p}") for p in range(PAIRS)]
        kT_sb = [qkpool.tile([P, S], bf16, tag=f"kT{p}", name=f"kT_sb{p}") for p in range(PAIRS)]
        vaug_sb = [vpool.tile([P, NH, DK + 1], bf16, tag=f"va{j}", name=f"vaug_sb{j}") for j in range(NT)]
        oT_sb = [otpool.tile([P, S], bf16, tag=f"oT{p}", name=f"oT_sb{p}") for p in range(PAIRS)]

        wv0 = 2 * DKH

        def proj_chain(dst, w0, x_t, bias, p, c):
            ps = pjps.tile([P, MC], f32, tag="pj", name="ps_kq")
            for i in range(ET):
                nc.tensor.matmul(
                    ps,
                    wkqv_sb[:, i, w0 + P * p : w0 + P * (p + 1)],
                    x_t[:, i, MC * c : MC * (c + 1)],
                    start=(i == 0),
                    stop=(i == ET - 1),
                )
            dslice = dst[p][:, MC * c : MC * (c + 1)]
            if bias is not None:
                nc.vector.tensor_scalar_add(dslice, ps, bias[:, p : p + 1])
            else:
                nc.vector.tensor_copy(dslice, ps)

        def v_chain(j):
            # v_aug[n-tile j] = xv-tile stationary, wv moving: [128 n, 256 dv]
            ps = pjps.tile([P, MC], f32, tag="pj", name="ps_v")
            pv = ps[:, 0:DKH]
            for i in range(ET):
                nc.tensor.matmul(
                    pv,
                    xv_t[:, i, P * j : P * (j + 1)],
                    wkqv_sb[:, i, wv0 : wv0 + DKH],
                    start=(i == 0),
                    stop=(i == ET - 1),
                )
            if bias_v:
                nc.vector.tensor_add(pv, pv, bv_sb)
            pt3 = pv.rearrange("n (h d) -> n h d", h=NH)
            nc.vector.tensor_copy(vaug_sb[j][:, :, 0:DK], pt3)
            nc.gpsimd.memset(vaug_sb[j][:, :, DK : DK + 1], 1.0)

        def op_chain(t, ec, evict_eng, dma_eng, pool_tag=None):
            pool, tag = pool_tag or (pjps, "pj")
            op = pool.tile([P, MC], f32, tag=tag, name="op_t")
            for p in range(PAIRS):
                nc.tensor.matmul(
                    op,
                    oT_sb[p][:, P * t : P * (t + 1)],
                    wo_sb[:, p, MC * ec : MC * (ec + 1)],
                    start=(p == 0),
                    stop=(p == PAIRS - 1),
                )
            ost = ostpool.tile([P, MC], bf16, tag="ost", name="ost_t")
            if evict_eng is nc.scalar:
                nc.scalar.copy(ost, op)
            else:
                evict_eng.tensor_copy(ost, op)
            dma_eng.dma_start(
                out=out[P * t : P * (t + 1), MC * ec : MC * (ec + 1)], in_=ost
            )

        def attn_group(c, p, fillers):
            """fillers: list of zero-arg callables emitting PE filler chains
            (v_aug builds, outproj chains); one is consumed right before
            each attnV step so the PE has work while ACT runs exp."""
            J = chunk_ntiles[c]
            oaug = [
                oaps.tile([P, MC], f32, tag="oaug", name=f"oaug{h01}")
                for h01 in range(2)
            ]
            probs_tiles = [None] * J

            def scores_step(j):
                # columns left of `off` in this m-chunk are fully masked
                # for n-tile j: never compute/exp/consume them
                off = P * (j - 4 * c) if (causal and j >= 4 * c) else 0
                sc = scps.tile([P, 2 * MC], f32, tag="sc", name="sc_ps_t")
                for h01 in range(2):
                    nc.tensor.matmul(
                        sc[:, MC * h01 + off : MC * (h01 + 1)],
                        kT_sb[p][64 * h01 : 64 * (h01 + 1), P * j : P * (j + 1)],
                        qT_sb[p][64 * h01 : 64 * (h01 + 1), MC * c + off : MC * (c + 1)],
                        start=True,
                        stop=True,
                    )
                probs = prpool.tile([P, 2 * MC], bf16, tag="probs", name="probs_t")
                sc3 = sc.rearrange("p (u m) -> p u m", u=2)
                pr3 = probs.rearrange("p (u m) -> p u m", u=2)
                nc.scalar.activation(
                    pr3[:, :, off:MC], sc3[:, :, off:MC], Exp, bias=0.0, scale=SCALE
                )
                if causal and j >= 4 * c:
                    for h01 in range(2):
                        base = MC * h01 + off
                        nc.gpsimd.tensor_mul(
                            probs[:, base : base + P],
                            probs[:, base : base + P],
                            dmask_sb,
                        )
                probs_tiles[j] = (probs, off)

            def attnv_step(j):
                probs, off = probs_tiles[j]
                for h01 in range(2):
                    h = 2 * p + h01
                    nc.tensor.matmul(
                        oaug[h01][0 : DK + 1, off:MC],
                        vaug_sb[j][:, h, :],
                        probs[:, MC * h01 + off : MC * (h01 + 1)],
                        start=(j == 0),
                        stop=(j == J - 1),
                    )

            # software pipeline: scores two steps ahead of attnV, with one
            # PE filler chain in front of each attnV step
            def take_filler():
                if fillers:
                    fillers.pop(0)()

            # depth-3 pipeline: attnV trails scores by 3 steps so it never
            # waits on the exp semaphore (scores themselves are 2-limited
            # by the sc PSUM rotation)
            for j in range(J):
                scores_step(j)
                if j >= 3:
                    take_filler()
                    attnv_step(j - 3)
            for j in range(max(0, J - 3), J):
                take_filler()
                attnv_step(j)

            # epilogue: evict o_aug to SBUF (frees PSUM); bounce the
            # denominator row once through DRAM to re-read it broadcast
            # across 64 partitions (step-0 partition AP), fast-reciprocal,
            # multiply into oT.
            # last group's bounce goes through scalar (idle after the final
            # exp) so it never queues behind out-DMAs on sync
            den_eng = nc.scalar if (c == NMC - 1 and p == PAIRS - 1) else nc.sync
            osb = []
            for h01 in range(2):
                o = rcpool.tile([DK + 1, MC], f32, tag="osb", bufs=4, name="osb_t")
                nc.vector.tensor_copy(o, oaug[h01][0 : DK + 1, :])
                den_d = rcdram.tile([1, MC], f32, tag="den_d", name="den_d_t")
                den_eng.dma_start(out=den_d, in_=o[DK : DK + 1, :])
                bcden = rcpool.tile([64, MC], f32, tag="bc", bufs=4, name="bc_t")
                den_eng.dma_start(
                    out=bcden,
                    in_=bass.AP(
                        tensor=den_d.tensor,
                        offset=den_d.offset,
                        ap=[[0, 64]] + [list(a) for a in den_d.ap[1:]],
                    ),
                )
                osb.append((o, bcden))
            for h01 in range(2):
                o, bcden = osb[h01]
                rc = rcpool.tile([64, MC], f32, tag="rc", bufs=2, name="rc_t")
                nc.vector.reciprocal_approx_fast(out=rc, in_=bcden)
                nc.vector.tensor_mul(
                    oT_sb[p][64 * h01 : 64 * (h01 + 1), MC * c : MC * (c + 1)],
                    o[0:DK, :],
                    rc,
                )

        # ---- schedule ----
        # Chunk-pipelined: k/q chains for chunk c+1 are emitted BETWEEN
        # attention windows (their DVE evictions land before the windows'
        # deferred epilogues, so they never queue behind a bounce DMA).
        # v_aug builds and the previous chunk's outproj chains ride inside
        # the attention groups as PE fillers, one per attnV step.
        kb = bk_sb if bias_qk else None
        qb = bq_sb if bias_qk else None

        def kq(c):
            for p in range(PAIRS):
                proj_chain(kT_sb, 0, xk_t, kb, p, c)
            for p in range(PAIRS):
                proj_chain(qT_sb, DKH, xq_t, qb, p, c)

        if causal:
            kq(0)
        else:
            # non-causal: attn(0) consumes every kT tile, so all k/q must
            # precede it in the in-order PE stream
            for c in range(NMC):
                kq(c)
        for c in range(NMC):
            if causal:
                vjs = list(range(4 * c, 4 * c + 4))
            else:
                vjs = list(range(NT)) if c == 0 else []
            fillers = [
                (lambda j=j: v_chain(j)) for j in vjs
            ]
            if c > 0:
                fillers += [
                    (lambda t=t, ec=ec: op_chain(t, ec, nc.vector, nc.sync))
                    for t in range(4 * (c - 1), 4 * c)
                    for ec in range(E // MC)
                ]
            if causal and c + 1 < NMC:
                # next chunk's k/q chains ride as late fillers too: exp(0)
                # starts ~20us earlier than with a full k/q prologue
                fillers += [
                    (lambda p=p, c1=c + 1: proj_chain(kT_sb, 0, xk_t, kb, p, c1))
                    for p in range(PAIRS)
                ]
                fillers += [
                    (lambda p=p, c1=c + 1: proj_chain(qT_sb, DKH, xq_t, qb, p, c1))
                    for p in range(PAIRS)
                ]
            for p in range(PAIRS):
                attn_group(c, p, fillers)
            for f in fillers:
                f()
            del fillers[:]

        # PE warmers: reader-less matmuls that execute inside the last
        # epilogue's ~6us dependency gap, holding the PE at full p-state so
        # the tail outproj streams at 2.4GHz instead of ramping from 1.2GHz
        for w in range(8):
            pool, tag = (pjps, "pj") if w % 2 == 0 else (oaps, "oaug")
            warm = pool.tile([P, MC], f32, tag=tag, name="warm_t")
            nc.tensor.matmul(
                warm,
                qT_sb[0][:, 0:P],
                qT_sb[1][:, 0:MC],
                start=True,
                stop=True,
            )

        # tail: outproj of the last chunk.  ACT is idle now (scalar evicts
        # half); chains alternate between the pj and oaug PSUM pools so four
        # banks rotate instead of two.
        n = 0
        tail_dma = (nc.sync, nc.gpsimd, nc.scalar)
        for t in range(4 * (NMC - 1), 4 * NMC):
            for ec in range(E // MC):
                op_chain(
                    t,
                    ec,
                    nc.scalar if n % 2 == 0 else nc.vector,
                    tail_dma[n % 3],
                    pool_tag=(pjps, "pj") if n % 2 == 0 else (oaps, "oaug"),
                )
                n += 1

    nc.compile()
    return nc


def _host_inputs(key, value, query, Wk, Wq, Wv, Wo, bq, bk, bv, bias_qk, bias_v):
    """Per-core input maps (host-side shard/transpose/cast — not timed)."""
    tri = np.triu(np.ones((P, P), np.float32)).astype(BF16)  # allowed: n<=m
    in_maps = []
    xT = {}
    for b in range(B):
        xqT = query[b].T.astype(BF16)  # [E, S]
        xkT = np.ascontiguousarray(key[b].T).astype(BF16)
        xvT = value[b].T.astype(BF16)
        # xqB[c, p, t*MC+n] = xqT[t*P+p, c*MC+n]  (m-chunk blocked)
        xT[("qB", b)] = np.ascontiguousarray(
            xqT.reshape(ET, P, NMC, MC).transpose(2, 1, 0, 3).reshape(NMC, P, ET * MC)
        )
        # xvB[nb, p, t*256+n] = xvT[t*P+p, nb*256+n]  (n-block blocked)
        xT[("vB", b)] = np.ascontiguousarray(
            xvT.reshape(ET, P, ET, 2 * P).transpose(2, 1, 0, 3).reshape(ET, P, ET * 2 * P)
        )
        xT[("k", b)] = xkT
    for c in range(NCORES):
        b, g = divmod(c, G)
        sl = slice(DKH * g, DKH * (g + 1))
        wkqv = np.concatenate(
            [Wk[sl].T, Wq[sl].T, Wv[sl].T], axis=1
        )  # [E, 3*DKH], column blocks K|Q|V
        m = {
            "xqB": xT[("qB", b)],
            "xkT": xT[("k", b)],
            "xvB": xT[("vB", b)],
            "wkqvT": np.ascontiguousarray(wkqv).astype(BF16),
            "woT": np.ascontiguousarray(Wo[:, sl].T).astype(BF16),
            "dmask": tri,
        }
        if bias_qk:
            m["bq"] = np.ascontiguousarray(bq[sl].astype(np.float32).reshape(DKH, 1))
            m["bk"] = np.ascontiguousarray(bk[sl].astype(np.float32).reshape(DKH, 1))
        if bias_v:
            m["bv"] = np.ascontiguousarray(bv[sl].astype(np.float32).reshape(1, DKH))
        in_maps.append(m)
    return in_maps


def _numpy_fallback(key, value, query, mask, Wk, bk, Wq, bq, Wv, bv, Wo, bo):
    """Exact reference semantics in numpy (general-mask fallback)."""
    def proj(x, W, b):
        return x @ W.T + b

    k = proj(key, Wk, bk).reshape(B, S, H, DK).transpose(0, 2, 1, 3)
    q = proj(query, Wq, bq).reshape(B, S, H, DK).transpose(0, 2, 1, 3)
    v = proj(value, Wv, bv).reshape(B, S, H, DK).transpose(0, 2, 1, 3)
    scores = np.einsum("bhmd,bhnd->bhmn", q, k).astype(np.float32)
    scores = np.where(mask, scores, np.float32(-1e10)) * np.float32(SCALE)
    scores -= scores.max(axis=3, keepdims=True)
    e = np.exp(scores)
    attn = e / e.sum(axis=3, keepdims=True)
    o = np.einsum("bhmn,bhnv->bhmv", attn, v)
    o = o.transpose(0, 2, 1, 3).reshape(B, S, E)
    return (o @ Wo.T + bo).astype(np.float32)


_program_cache = {}


def kernel(key, value, query, mask, Wk, bk, Wq, bq, Wv, bv, Wo, bo):
    key = np.asarray(key, np.float32)
    value = np.asarray(value, np.float32)
    query = np.asarray(query, np.float32)
    mask = np.asarray(mask)
    Wk, bk = np.asarray(Wk, np.float32), np.asarray(bk, np.float32)
    Wq, bq = np.asarray(Wq, np.float32), np.asarray(bq, np.float32)
    Wv, bv = np.asarray(Wv, np.float32), np.asarray(bv, np.float32)
    Wo, bo = np.asarray(Wo, np.float32), np.asarray(bo, np.float32)

    m2 = mask.reshape(B, S, S) if mask.size == B * S * S else None
    causal = m2 is not None and all(
        np.array_equal(m2[b], np.tril(np.ones((S, S), bool))) for b in range(B)
    )
    allones = m2 is not None and bool(mask.all())
    if not causal and not allones:
        return _numpy_fallback(key, value, query, mask, Wk, bk, Wq, bq, Wv, bv, Wo, bo)

    if causal:
        chunk_ntiles = tuple(4 * (c + 1) for c in range(NMC))
    else:
        chunk_ntiles = tuple(NT for _ in range(NMC))

    bias_qk = bool(np.any(bq) or np.any(bk))
    bias_v = bool(np.any(bv))

    pkey = (chunk_ntiles, causal, bias_qk, bias_v)
    if pkey not in _program_cache:
        _program_cache[pkey] = _build_program(chunk_ntiles, causal, bias_qk, bias_v)
    nc = _program_cache[pkey]

    from concourse.bass_utils import run_bass_kernel_spmd

    in_maps = _host_inputs(key, value, query, Wk, Wq, Wv, Wo, bq, bk, bv, bias_qk, bias_v)
    res = run_bass_kernel_spmd(nc, in_maps, core_ids=list(range(NCORES)))

    outp = np.zeros((B, S, E), np.float32)
    for c in range(NCORES):
        outp[c // G] += np.asarray(res.results[c]["out"], np.float32)
    outp += bo.astype(np.float32)
    return outp
